# revision 1
# baseline (speedup 1.0000x reference)
"""Cross-attention Bass kernel for 8 trn2 NeuronCores — v2 (bf16 + DMA-crossbar transposes + natural-PV + pipelined emission).

Sharding: core d handles batch b = d//4, query rows [(d%4)*1024, ...+1024),
all 8 heads (no collectives). Context compacted on host via mask, padded to
m_pad = ceil(max_meff/128)*128 (seed-0 inputs: 2056 -> 2176, 17 k-tiles).

v2 strategy (vs baseline):
- Everything bf16 on the PE (1.0 cyc/row in the cost model; halves DMA+SBUF).
- x^T, ctx^T, O^T produced by DMA crossbar transposes (dma_start_transpose,
  14ns per 16x128 tile on the DMA engines) — zero PE/DVE cost, no PSUM.
- PV in NATURAL orientation: out[q_chunk(128), 65] per (head, k-tile):
  stationary = P^T slice, moving = [V | valid] per head; free=65 instead of
  512 cuts PV from 147k to 71k PE cycles. Column 64 accumulates the softmax
  denominator. One accumulation chain per PSUM bank at a time
  (start_tensor_calc zeroes the whole 2KB bank), ring-alternating two banks.
- ACT does ONLY exps (PSUM f32 -> SBUF bf16 P^T); DVE handles all other
  drains, the normalize (reciprocal + broadcast mul), and the bias add.
- Software-pipelined emission: unit u's PV chains are interleaved between
  unit u+1's score groups so the PE never drains and ACT always has scores
  queued. The K/V projection prologue is emitted block-by-block with the
  first unit's score groups interleaved.
- PSUM: tag "sc" [128,1536] f32 x2 (6 banks) + tag "pv" [128,512] f32 x2
  (2 banks) = 8 banks exactly.
"""
import numpy as np

B, N, M = 2, 4096, 4096
QUERY_DIM, CONTEXT_DIM = 512, 768
H, D = 8, 64
INNER = H * D  # 512
NCORES = 8
N_DEV = (B * N) // NCORES  # 1024 query rows per core
QB = 512
NQB = N_DEV // QB  # 2
SCALE = float(D) ** -0.5

_compiled = {}


def _build(m_pad):
    from concourse import bacc
    import concourse.bass as bass
    import concourse.mybir as mybir
    import concourse.tile as tile

    F32 = mybir.dt.float32
    BF = mybir.dt.bfloat16
    AF = mybir.ActivationFunctionType

    T = m_pad // 128  # k-tiles
    PBUFS = 6 if T <= 17 else (5 if T <= 19 else 3)
    MBLK = [(s, min(512, m_pad - s)) for s in range(0, m_pad, 512)]
    SC_G = 3
    GROUPS = [(g, min(SC_G, T - g)) for g in range(0, T, SC_G)]

    CQ = QUERY_DIM // 128  # 4
    CC = CONTEXT_DIM // 128  # 6
    CI = INNER // 128  # 4

    nc = bacc.Bacc()
    xs_d = nc.declare_dram_parameter("xs", [N_DEV, QUERY_DIM], BF, isOutput=False)
    ctx_d = nc.declare_dram_parameter("ctx", [m_pad, CONTEXT_DIM], BF, isOutput=False)
    val_d = nc.declare_dram_parameter("valid", [m_pad], BF, isOutput=False)
    wq_d = nc.declare_dram_parameter("Wq", [QUERY_DIM, INNER], BF, isOutput=False)
    wk_d = nc.declare_dram_parameter("Wk", [CONTEXT_DIM, INNER], BF, isOutput=False)
    wv_d = nc.declare_dram_parameter("Wv", [CONTEXT_DIM, INNER], BF, isOutput=False)
    wo_d = nc.declare_dram_parameter("Wo", [INNER, QUERY_DIM], BF, isOutput=False)
    bo_d = nc.declare_dram_parameter("bo", [QUERY_DIM], F32, isOutput=False)
    out_d = nc.declare_dram_parameter("out", [N_DEV, QUERY_DIM], F32, isOutput=True)

    with tile.TileContext(nc) as tc:
        with (
            tc.tile_pool(name="big", bufs=1) as big,
            tc.tile_pool(name="ctxt", bufs=2) as ctxt,
            tc.tile_pool(name="pb", bufs=PBUFS) as pbp,
            tc.tile_pool(name="sm", bufs=4) as sm,
            tc.tile_pool(name="tay", bufs=2) as tay,
            tc.tile_pool(name="outp", bufs=2) as outp,
            tc.tile_pool(name="ps_sc", bufs=2, space="PSUM") as ps_sc,
            tc.tile_pool(name="ps_pv", bufs=2, space="PSUM") as ps_pv,
        ):
            # ---- persistent SBUF tensors ----
            wo = big.tile([128, CI, QUERY_DIM], BF, tag="wo", name="wo")
            bo_bc = big.tile([128, QUERY_DIM], F32, tag="bo", name="bo")
            valid = big.tile([128, T], BF, tag="valid", name="valid")
            qT = big.tile([128, CI, N_DEV], BF, tag="qT", name="qT")
            kT = big.tile([128, CI, m_pad], BF, tag="kT", name="kT")
            v2 = big.tile([128, T, H, 65], BF, tag="v2", name="v2")
            onat = [
                big.tile([128, CI, H, 64], BF, tag=f"onat{qb}", name=f"onat{qb}")
                for qb in range(NQB)
            ]
            oT = [
                big.tile([128, CI, QB], BF, tag=f"oT{qb}", name=f"oT{qb}")
                for qb in range(NQB)
            ]

            nc.sync.dma_start(
                out=valid[:], in_=val_d[:].rearrange("(t p) -> p t", p=128)
            )
            nc.sync.dma_start(
                out=bo_bc[:],
                in_=bass.AP(tensor=bo_d, offset=0, ap=[[0, 128], [1, QUERY_DIM]]),
            )

            wq = big.tile([128, CQ, INNER], BF, tag="wq", name="wq")
            wk = big.tile([128, CC, INNER], BF, tag="wk", name="wk")
            wv = big.tile([128, CC, INNER], BF, tag="wv", name="wv")
            xT = big.tile([128, CQ, N_DEV], BF, tag="xT", name="xT")

            nc.gpsimd.dma_start(
                out=wq[:], in_=wq_d[:].rearrange("(o p) f -> p o f", p=128)
            )
            nc.gpsimd.dma_start(
                out=wk[:], in_=wk_d[:].rearrange("(o p) f -> p o f", p=128)
            )
            nc.gpsimd.dma_start(
                out=wv[:], in_=wv_d[:].rearrange("(o p) f -> p o f", p=128)
            )
            nc.gpsimd.dma_start(
                out=wo[:], in_=wo_d[:].rearrange("(o p) f -> p o f", p=128)
            )
            # x^T straight from DRAM via the DMA crossbar
            nc.sync.dma_start_transpose(out=xT[:], in_=xs_d[:])

            # ---- prologue pieces ----
            def emit_q_proj():
                for dc in range(CI):
                    for qf in range(N_DEV // 512):
                        psq = ps_pv.tile([128, 512], F32, tag="pv", name="psq")
                        for c in range(CQ):
                            nc.tensor.matmul(
                                psq[:],
                                wq[:, c, dc * 128 : (dc + 1) * 128],
                                xT[:, c, qf * 512 : (qf + 1) * 512],
                                start=(c == 0),
                                stop=(c == CQ - 1),
                            )
                        nc.vector.tensor_scalar_mul(
                            qT[:, dc, qf * 512 : (qf + 1) * 512], psq[:], SCALE
                        )

            def emit_ctx_block(bi):
                base, bw = MBLK[bi]
                ctxT = ctxt.tile([128, CC, 512], BF, tag="ctxT", name="ctxT")
                nc.sync.dma_start_transpose(
                    out=ctxT[:, :, 0:bw], in_=ctx_d[base : base + bw, :]
                )
                return ctxT

            def emit_k_block(bi, ctxT):
                base, bw = MBLK[bi]
                for dc in range(CI):
                    psk = ps_pv.tile([128, 512], F32, tag="pv", name="psk")
                    for c in range(CC):
                        nc.tensor.matmul(
                            psk[:, :bw],
                            wk[:, c, dc * 128 : (dc + 1) * 128],
                            ctxT[:, c, :bw],
                            start=(c == 0),
                            stop=(c == CC - 1),
                        )
                    nc.vector.tensor_copy(kT[:, dc, base : base + bw], psk[:, :bw])

            def emit_v_block(bi, ctxT):
                base, bw = MBLK[bi]
                for ktl in range(bw // 128):
                    t = base // 128 + ktl
                    psv = ps_pv.tile([128, 512], F32, tag="pv", name="psv")
                    for c in range(CC):
                        nc.tensor.matmul(
                            psv[:],
                            ctxT[:, c, ktl * 128 : (ktl + 1) * 128],
                            wv[:, c, :],
                            start=(c == 0),
                            stop=(c == CC - 1),
                        )
                    nc.vector.tensor_copy(
                        v2[:, t, :, 0:64],
                        psv[:].rearrange("p (h d) -> p h d", d=64),
                    )
                    nc.vector.tensor_copy(
                        v2[:, t, :, 64:65],
                        valid[:, t : t + 1].to_broadcast([128, H, 1]),
                    )

            # ---- attention unit pieces ----
            def unit_scores_group(qb, h, g0, gn, pb, taylor=False):
                dc = h // 2
                half = (h % 2) * 64
                sc = ps_sc.tile([128, 1536], F32, tag="sc", name="sc")
                for j in range(gn):
                    t = g0 + j
                    nc.tensor.matmul(
                        sc[:, j * 512 : (j + 1) * 512],
                        kT[half : half + 64, dc, t * 128 : (t + 1) * 128],
                        qT[half : half + 64, dc, qb * QB : (qb + 1) * QB],
                        start=True,
                        stop=True,
                    )
                scv = sc[:, 0 : gn * 512].rearrange("p (g q) -> p g q", q=512)
                if not taylor:
                    nc.scalar.activation(pb[:, g0 : g0 + gn, :], scv, AF.Exp)
                else:
                    # scores are tiny (|s| <~ 1.2): exp(s) ~ 0.5*(s+1)^2 + 0.5,
                    # computed on DVE+Pool to offload the ACT engine
                    t1 = tay.tile([128, SC_G, 512], BF, tag="t1", name="t1")
                    t2 = tay.tile([128, SC_G, 512], BF, tag="t2", name="t2")
                    nc.vector.tensor_scalar_add(t1[:, 0:gn, :], scv, 1.0)
                    nc.gpsimd.tensor_mul(t2[:, 0:gn, :], t1[:, 0:gn, :], t1[:, 0:gn, :])
                    nc.vector.tensor_scalar(
                        pb[:, g0 : g0 + gn, :],
                        t2[:, 0:gn, :],
                        0.5,
                        0.5,
                        mybir.AluOpType.mult,
                        mybir.AluOpType.add,
                    )

            def unit_pv_chain(qb, h, c, pb):
                pv = ps_pv.tile([128, 512], F32, tag="pv", name="pv")
                for t in range(T):
                    nc.tensor.matmul(
                        pv[:, 0:65],
                        pb[:, t, c * 128 : (c + 1) * 128],
                        v2[:, t, h, :],
                        start=(t == 0),
                        stop=(t == T - 1),
                    )
                linv = sm.tile([128, 1], F32, tag="linv", name="linv")
                nc.vector.reciprocal(linv[:], pv[:, 64:65])
                nc.vector.tensor_mul(
                    onat[qb][:, c, h, :],
                    pv[:, 0:64],
                    linv[:].to_broadcast([128, 64]),
                )

            def emit_oT(qb):
                for qtl in range(CI):
                    nc.sync.dma_start_transpose(
                        out=oT[qb][:, :, qtl * 128 : (qtl + 1) * 128],
                        in_=onat[qb][:, qtl, :, :],
                    )

            def out_proj_chain(qb, qtl):
                pso = ps_pv.tile([128, 512], F32, tag="pv", name="pso")
                for c in range(CI):
                    nc.tensor.matmul(
                        pso[:],
                        oT[qb][:, c, qtl * 128 : (qtl + 1) * 128],
                        wo[:, c, :],
                        start=(c == 0),
                        stop=(c == CI - 1),
                    )
                ot = outp.tile([128, QUERY_DIM], F32, tag="ot", name="ot")
                nc.vector.tensor_add(ot[:], pso[:], bo_bc[:])
                qt = qb * (QB // 128) + qtl
                nc.sync.dma_start(out=out_d[qt * 128 : (qt + 1) * 128, :], in_=ot[:])

            # ---- emission schedule (software pipeline) ----
            UNITS = [(qb, h) for qb in range(NQB) for h in range(H)]
            NU = len(UNITS)
            TAYLOR_UNITS = set()
            PRO = PBUFS  # units whose scores interleave into the prologue
            pbs = {}
            done_g = {}  # unit -> next un-emitted score-group index

            for ui in range(PRO):
                pbs[ui] = pbp.tile([128, T, 512], BF, tag="pb", name="pb")
                done_g[ui] = 0

            emit_q_proj()

            # prologue blocks with the first PRO units' score groups greedy
            deferred_v = []
            tiles_ready = 0
            for bi in range(len(MBLK)):
                ctxT = emit_ctx_block(bi)
                emit_k_block(bi, ctxT)
                for ui in range(PRO):
                    qb, h = UNITS[ui]
                    tr = tiles_ready + MBLK[bi][1] // 128
                    while (
                        done_g[ui] < len(GROUPS)
                        and GROUPS[done_g[ui]][0] + GROUPS[done_g[ui]][1] <= tr
                    ):
                        g0, gn = GROUPS[done_g[ui]]
                        unit_scores_group(qb, h, g0, gn, pbs[ui])
                        done_g[ui] += 1
                if bi == 0:
                    emit_v_block(bi, ctxT)
                else:
                    deferred_v.append(bi)
                tiles_ready += MBLK[bi][1] // 128

            # steady state: PV of unit u first (freeing its pb ring slot),
            # then unit u+LAG's score groups into the freed slot. This lets
            # all PBUFS ring slots host prologue units (PRO == PBUFS).
            LAG = PRO
            for u in range(NU):
                qb, h = UNITS[u]
                if u == 0:
                    # deferred V blocks must land before the first PV chain
                    while deferred_v:
                        bi = deferred_v.pop(0)
                        emit_v_block(bi, emit_ctx_block(bi))
                for c in range(4):
                    unit_pv_chain(qb, h, c, pbs[u])
                del pbs[u]
                if h == H - 1:
                    emit_oT(qb)
                    for qtl in range(CI):
                        out_proj_chain(qb, qtl)
                fu = u + LAG
                if fu < NU:
                    fqb, fh = UNITS[fu]
                    pbs[fu] = pbp.tile([128, T, 512], BF, tag="pb", name="pb")
                    for g0, gn in GROUPS:
                        unit_scores_group(fqb, fh, g0, gn, pbs[fu])

    nc.compile()
    return nc


def kernel(x, context_tensor, mask, Wq, Wk, Wv, Wo, bo):
    import ml_dtypes
    from concourse.bass_utils import run_bass_kernel_spmd

    BF = ml_dtypes.bfloat16
    x = np.asarray(x, dtype=np.float32)
    context_tensor = np.asarray(context_tensor, dtype=np.float32)
    mask = np.asarray(mask)
    Wq = np.asarray(Wq, dtype=np.float32).astype(BF)
    Wk = np.asarray(Wk, dtype=np.float32).astype(BF)
    Wv = np.asarray(Wv, dtype=np.float32).astype(BF)
    Wo = np.asarray(Wo, dtype=np.float32).astype(BF)
    bo = np.ascontiguousarray(np.asarray(bo, dtype=np.float32))

    # host-side context compaction using the mask
    meffs = [int(mask[b].sum()) for b in range(B)]
    m_pad = ((max(meffs) + 127) // 128) * 128
    ctx_c = np.zeros((B, m_pad, CONTEXT_DIM), dtype=BF)
    val = np.zeros((B, m_pad), dtype=BF)
    for b in range(B):
        idx = np.flatnonzero(mask[b])
        ctx_c[b, : len(idx)] = context_tensor[b, idx].astype(BF)
        val[b, : len(idx)] = 1.0
    xb = x.astype(BF)

    if m_pad not in _compiled:
        _compiled[m_pad] = _build(m_pad)
    nc = _compiled[m_pad]

    rows_per_core = N // (NCORES // B)  # 1024
    in_maps = []
    for d in range(NCORES):
        b = d // (NCORES // B)
        r0 = (d % (NCORES // B)) * rows_per_core
        in_maps.append(
            {
                "xs": xb[b, r0 : r0 + rows_per_core],
                "ctx": ctx_c[b],
                "valid": val[b],
                "Wq": Wq,
                "Wk": Wk,
                "Wv": Wv,
                "Wo": Wo,
                "bo": bo,
            }
        )

    res = run_bass_kernel_spmd(nc, in_maps, list(range(NCORES)))
    out = np.empty((B, N, QUERY_DIM), dtype=np.float32)
    for d in range(NCORES):
        b = d // (NCORES // B)
        r0 = (d % (NCORES // B)) * rows_per_core
        out[b, r0 : r0 + rows_per_core] = res.results[d]["out"]
    return out

